# revision 36
# baseline (speedup 1.0000x reference)
"""Self-contained TRN2 Bass kernel for nn_AESModel_42760694399363.

2-layer NF4-quantized transformer (B=4,S=1024,D=2048,FF=8192,H=16) + mean-pool
+ linear head.  Tensor-parallel across 8 NeuronCores: q/k/v/gate/up
column-sharded, o/down row-sharded.  The residual stream is sequence-sharded
STRIDED BY BATCH: core c owns tokens {1024*q + 128*c .. +128} for each batch
q.  This makes every boundary collective a per-batch 4MB ReduceScatter /
AllGather that pipelines behind the next batch's attention (or next batch's
MLP chunk), hiding nearly all collective latency.  Host does embedding gather
and NF4 dequant; all matmul/attention/norm FLOPs run on device in bf16 with
f32 accumulation.
"""
import sys

sys.path.insert(0, "/opt/trn_rl_repo")

import numpy as np
import ml_dtypes

# ---------------------------------------------------------------- constants
B, S, D, L, FF, V, NS = 4, 1024, 2048, 2, 8192, 32000, 11
H, DH = 16, 128
NCORES = 8
TOK = B * S              # 4096
SHARD = TOK // NCORES    # 512 tokens per core (4 tiles of 128, one per batch)
DQ = D // NCORES         # 256  q/k/v out-cols per core
FFC = FF // NCORES       # 1024 gate/up cols per core
KC = D // 128            # 16 contraction chunks over D
SCALE = 1.0 / np.sqrt(DH)
EPS = 1e-5
ROPE_THETA = 10000.0
BLK = 64

NF4 = np.array([-1.0, -0.6961928009986877, -0.5250730514526367, -0.39491748809814453,
                -0.28444138169288635, -0.18477343022823334, -0.09105003625154495, 0.0,
                0.07958029955625534, 0.16093020141124725, 0.24611230194568634,
                0.33791524171829224, 0.44070982933044434, 0.5626170039176941,
                0.7229568362236023, 1.0], dtype=np.float32)

BF = ml_dtypes.bfloat16

_CACHE = {}


# ---------------------------------------------------------------- device graph
def build_graph():
    import concourse.mybir as mybir
    import concourse.tile as tile
    from concourse import bacc
    from concourse.masks import make_identity
    from concourse.tile_rust import add_dep_helper

    F32 = mybir.dt.float32
    BF16 = mybir.dt.bfloat16
    RG = [list(range(NCORES))]
    Exp = mybir.ActivationFunctionType.Exp
    Sigmoid = mybir.ActivationFunctionType.Sigmoid

    nc = bacc.Bacc("TRN2", target_bir_lowering=False, debug=False,
                   num_devices=NCORES)
    _prev_cc = [None]

    def _chain_cc(cc):
        # never allow two collectives in flight: serialize in issue order
        if _prev_cc[0] is not None:
            add_dep_helper(cc.ins, _prev_cc[0], reason="serialize collectives")
        _prev_cc[0] = cc.ins

    x_ext = nc.declare_dram_parameter("x", [SHARD, D], F32, isOutput=False)
    hall0_ext = nc.declare_dram_parameter("hall0", [B, NCORES * 128, D], BF16,
                                          isOutput=False)
    wq_ext = nc.declare_dram_parameter("wq", [L, 128, KC * DQ], BF16, isOutput=False)
    wk_ext = nc.declare_dram_parameter("wk", [L, 128, KC * DQ], BF16, isOutput=False)
    wv_ext = nc.declare_dram_parameter("wv", [L, 128, KC * DQ], BF16, isOutput=False)
    wo_ext = nc.declare_dram_parameter("wo", [L, 128, 2 * D], BF16, isOutput=False)
    wg_ext = nc.declare_dram_parameter("wg", [L, 128, KC * FFC], BF16, isOutput=False)
    wu_ext = nc.declare_dram_parameter("wu", [L, 128, KC * FFC], BF16, isOutput=False)
    wd_ext = nc.declare_dram_parameter("wd", [L, 128, 8 * D], BF16, isOutput=False)
    cosf_ext = nc.declare_dram_parameter("cosf", [128, S], BF16, isOutput=False)
    sinf_ext = nc.declare_dram_parameter("sinf", [128, S], BF16, isOutput=False)
    cm_ext = nc.declare_dram_parameter("cmask", [4, 128, 512], BF16, isOutput=False)
    hw_ext = nc.declare_dram_parameter("hw", [D, NS], F32, isOutput=False)
    out_ext = nc.declare_dram_parameter("out", [NS, B], F32, isOutput=True)

    with tile.TileContext(nc) as tc:
        with tc.tile_pool(name="const", bufs=1) as constp, \
             tc.tile_pool(name="xres", bufs=1) as xres, \
             tc.tile_pool(name="norm", bufs=1) as normp, \
             tc.tile_pool(name="wpre", bufs=1) as wpre, \
             tc.tile_pool(name="hpool", bufs=2) as hpool, \
             tc.tile_pool(name="small", bufs=4) as small, \
             tc.tile_pool(name="drain", bufs=8) as drain, \
             tc.tile_pool(name="psmm", bufs=5, space="PSUM") as psmm, \
             tc.tile_pool(name="pstr", bufs=2, space="PSUM") as pstr, \
             tc.tile_pool(name="psq", bufs=1, space="PSUM") as psq, \
             tc.tile_pool(name="dram", bufs=1, space="DRAM") as dram:

            warm_in = dram.tile([8, 8], BF16, tag="ccwarm_in", name="ccwarm_in")
            warm_out = dram.tile([64, 8], BF16, tag="ccwarm_out",
                                 name="ccwarm_out", addr_space="Shared")
            cc0 = nc.gpsimd.collective_compute(
                "AllGather", mybir.AluOpType.bypass,
                replica_groups=[list(range(NCORES))],
                ins=[warm_in.opt()], outs=[warm_out.opt()])
            _chain_cc(cc0)
            ident = constp.tile([128, 128], BF16, tag="ident")
            make_identity(nc, ident[:])
            hw_sb = constp.tile([128, KC, NS], F32, tag="hw")
            nc.scalar.dma_start(
                hw_sb[:], hw_ext.ap().rearrange("(c p) m -> p c m", p=128))
            pvec = constp.tile([128, 128], BF16, tag="pvec")
            nc.vector.memset(pvec[:], 1.0 / S)
            pooled1 = constp.tile([1, D], F32, tag="pooled1")
            pooledT = constp.tile([128, KC, 4], F32, tag="pooledT")

            # residual stream, f32, [128 part, q=batch tile, D]
            x_sb = xres.tile([128, 4, D], F32, tag="x")

            def rmsnorm_bf16(t, h_out):
                """h_out[128, D] bf16 = x_sb[:, t, :] * rsqrt(mean(x^2)+eps)."""
                ssq = small.tile([128, 1], F32, tag="ssq")
                nc.scalar.activation(h_out[:], x_sb[:, t, :],
                                     mybir.ActivationFunctionType.Square,
                                     accum_out=ssq[:])
                ms = small.tile([128, 1], F32, tag="ms")
                nc.vector.tensor_scalar(ms[:], ssq[:], 1.0 / D, EPS,
                                        mybir.AluOpType.mult,
                                        mybir.AluOpType.add)
                st = small.tile([128, 1], F32, tag="st")
                nc.scalar.sqrt(st[:], ms[:])
                rstd = small.tile([128, 1], F32, tag="rstd")
                nc.vector.reciprocal(rstd[:], st[:])
                nc.vector.tensor_scalar_mul(h_out[:], x_sb[:, t, :], rstd[:])

            def norm_transpose(t, hct_q):
                """rmsnorm batch-tile t -> PE transposes -> hct_q [128, (c n)]."""
                hbf = normp.tile([128, D], BF16, tag="hbf")
                rmsnorm_bf16(t, hbf)
                for kq in range(KC // 4):
                    pt = pstr.tile([128, 4, 128], BF16, tag="tr")
                    for q in range(4):
                        nc.tensor.transpose(
                            pt[:, q, :],
                            hbf[:, 512 * kq + 128 * q:512 * kq + 128 * (q + 1)],
                            ident[:])
                    hts = drain.tile([128, 4, 128], BF16, tag="ob")
                    nc.vector.tensor_copy(hts[:], pt[:])
                    nc.sync.dma_start(
                        hct_q[:, 512 * kq:512 * (kq + 1)], hts[:])

            def allgather_q(hct_q, tagsuf):
                """AG [128, D] -> [NCORES*128, D] (4MB)."""
                hall_q = dram.tile([NCORES * 128, D], BF16, tag="hall" + tagsuf,
                                   name="hall" + tagsuf, addr_space="Shared")
                cc = nc.gpsimd.collective_compute(
                    "AllGather", mybir.AluOpType.bypass, replica_groups=RG,
                    ins=[hct_q.opt()], outs=[hall_q.opt()])
                _chain_cc(cc)
                return hall_q

            def reduce_scatter_q(rs_in_q, tagsuf, cols=D):
                """RS [8*128, cols] -> [128, cols]."""
                rs_out_q = dram.tile([128, cols], BF16, tag="rso" + tagsuf,
                                     name="rso" + tagsuf)
                cc = nc.gpsimd.collective_compute(
                    "ReduceScatter", mybir.AluOpType.add, replica_groups=RG,
                    ins=[rs_in_q.opt()], outs=[rs_out_q.opt()])
                _chain_cc(cc)
                return rs_out_q

            def residual_add(t, rs_out_q):
                """x_sb[:, t, :] += rs_out_q [128, D] (bf16 in DRAM)."""
                for n in range(4):
                    db = drain.tile([128, 512], BF16, tag="ob")
                    nc.sync.dma_start(
                        db[:], rs_out_q[:, 512 * n:512 * (n + 1)])
                    nc.vector.tensor_add(
                        x_sb[:, t, 512 * n:512 * (n + 1)],
                        x_sb[:, t, 512 * n:512 * (n + 1)], db[:])

            def load_h_half(hall_q, rr):
                """Half of a batch's gathered h: [128, KC, 512] from hall_q."""
                h_sb = hpool.tile([128, KC, 512], BF16, tag="hr")
                for r in range(4):
                    rg = 4 * rr + r
                    nc.sync.dma_start(
                        h_sb[:, :, 128 * r:128 * (r + 1)],
                        hall_q[128 * rg:128 * (rg + 1), :].rearrange(
                            "p (c n) -> p c n", n=128))
                return h_sb

            def load_h_pair(hall_q):
                return [load_h_half(hall_q, 0), load_h_half(hall_q, 1)]

            pending = [None]

            def flush():
                if pending[0] is not None:
                    f = pending[0]
                    pending[0] = None
                    f()

            def make_post(b, rs_out_q, hct_name, agsuf, store):
                """Deferred post-RS work: residual add (+ norm/transpose/AG)."""
                def run():
                    residual_add(b, rs_out_q)
                    if store is not None:
                        hct_q = dram.tile([128, D], BF16, tag=hct_name,
                                          name=hct_name)
                        norm_transpose(b, hct_q)
                        store[b] = allgather_q(hct_q, agsuf)
                    else:
                        final_tile(b)
                return run

            def final_tile(t):
                """Final rmsnorm + mean-pool partial for batch tile t."""
                hfin = normp.tile([128, D], BF16, tag="hbf")
                rmsnorm_bf16(t, hfin)
                for n in range(4):
                    pq = psq.tile([128, 512], F32, tag="trf",
                                  name=f"pool{t}_{n}")
                    nc.tensor.matmul(pq[:], pvec[:],
                                     hfin[:, 512 * n:512 * (n + 1)],
                                     start=True, stop=True)
                    nc.vector.tensor_copy(
                        pooled1[0:1, 512 * n:512 * (n + 1)], pq[0:1, :])
                pdram_t = dram.tile([1, D], F32, tag=f"pd{t}", name=f"pd{t}")
                nc.sync.dma_start(pdram_t[:], pooled1[:])
                nc.sync.dma_start(
                    pooledT[:, :, t],
                    pdram_t[0:1, :].rearrange("a (c p) -> p (a c)", p=128))

            # ---------------- initial hidden state comes pre-normed,
            # pre-transposed and replicated from the host (hall0): no
            # initial AllGather chain on the critical path.
            hall = {t: hall0_ext[t] for t in range(4)}
            h_next = [load_h_pair(hall[0])]

            for l in range(L):
                # ======================= attention =======================
                with tc.tile_pool(name=f"attn{l}", bufs=1) as ap, \
                     tc.tile_pool(name=f"attn2{l}", bufs=1) as ap2, \
                     tc.tile_pool(name=f"attnpT{l}", bufs=1) as appt:
                    wq_sb = ap.tile([128, KC, DQ], BF16, tag="wq")
                    nc.scalar.dma_start(wq_sb[:], wq_ext[l])
                    wk_sb = ap.tile([128, KC, DQ], BF16, tag="wk")
                    nc.scalar.dma_start(wk_sb[:], wk_ext[l])
                    wv_sb = ap.tile([128, KC, DQ], BF16, tag="wv")
                    nc.scalar.dma_start(wv_sb[:], wv_ext[l])
                    cosf = ap.tile([128, S], BF16, tag="cosf")
                    nc.scalar.dma_start(cosf[:], cosf_ext[:])
                    sinf = ap.tile([128, S], BF16, tag="sinf")
                    nc.scalar.dma_start(sinf[:], sinf_ext[:])
                    cmask = [ap.tile([128, 512], BF16, tag=f"cmask{d}",
                                     name=f"cmask{l}_{d}") for d in range(4)]
                    for d in range(4):
                        nc.scalar.dma_start(cmask[d][:], cm_ext[d])
                    # prefetch next MLP's gate weights during attention
                    wg_sb = wpre.tile([128, KC, FFC], BF16, tag="wg")
                    nc.scalar.dma_start(wg_sb[:], wg_ext[l])
                    wo_sb = ap.tile([128, 2, D], BF16, tag="wo")
                    nc.scalar.dma_start(wo_sb[:], wo_ext[l])
                    if l == 0:
                        nc.sync.dma_start(
                            x_sb[:],
                            x_ext.ap().rearrange("(t p) d -> p t d", p=128))

                    hall_m = {}
                    for b in range(B):
                        h01 = h_next[0]
                        # ---- qkv for this batch (1024 tokens, 2 halves)
                        qTb = ap.tile([128, 2, S], BF16, tag="qTb")
                        kTb = ap.tile([128, 2, S], BF16, tag="kTb")
                        v_b = ap.tile([128, 8, DQ], BF16, tag="vb")
                        for rr in range(2):
                            hh_sb = h01[rr]
                            cs = cosf[:, 512 * rr:512 * (rr + 1)]
                            sn = sinf[:, 512 * rr:512 * (rr + 1)]
                            for wsb, dstT in ((wq_sb, qTb), (wk_sb, kTb)):
                                for m in range(2):
                                    pq = psmm.tile([128, 512], F32, tag="mm")
                                    for kc in range(KC):
                                        nc.tensor.matmul(
                                            pq[:],
                                            wsb[:, kc, 128 * m:128 * (m + 1)],
                                            hh_sb[:, kc, :],
                                            start=(kc == 0), stop=(kc == KC - 1))
                                    ta = ap2.tile([128, 512], F32, tag="ropea")
                                    nc.vector.tensor_mul(ta[:], pq[:], cs)
                                    tb = ap2.tile([128, 512], F32, tag="ropeb")
                                    nc.vector.tensor_mul(
                                        tb[0:64, :], pq[64:128, :], sn[0:64, :])
                                    nc.vector.tensor_mul(
                                        tb[64:128, :], pq[0:64, :],
                                        sn[64:128, :])
                                    nc.vector.tensor_add(
                                        dstT[:, m, 512 * rr:512 * (rr + 1)],
                                        ta[:], tb[:])
                            for t in range(4):
                                pv = psmm.tile([128, DQ], F32, tag="mm")
                                for kc in range(KC):
                                    nc.tensor.matmul(
                                        pv[:],
                                        hh_sb[:, kc, 128 * t:128 * (t + 1)],
                                        wv_sb[:, kc, :],
                                        start=(kc == 0), stop=(kc == KC - 1))
                                nc.vector.tensor_copy(
                                    v_b[:, 4 * rr + t, :], pv[:])
                        if b + 1 < B:
                            h_next[0] = load_h_pair(hall[b + 1])
                        else:
                            h_next[0] = load_h_pair(hall_m[0])
                        # ---- attention for b, per local head
                        oT = ap.tile([128, 2, S], BF16, tag="oT")
                        for hh in range(2):
                            if hh == 1:
                                flush()
                            qh = qTb[:, hh, :]
                            kh = kTb[:, hh, :]
                            pT_sb = appt.tile([128, 8, S], BF16, tag="pT")
                            for i in range(8):
                                nv = 512 if i < 4 else 1024
                                # scores stay in PSUM; causal mask accumulated
                                # by the PE (ident.T @ cmask); exp reads PSUM.
                                # No max-subtraction: |SCALE*s| << 80 for this
                                # model, so plain exp is exact & overflow-free.
                                pb = ap2.tile([128, S], BF16, tag="pb", bufs=3)
                                ses = []
                                for j in range(nv // 512):
                                    ps_s = psmm.tile([128, 512], F32, tag="mm")
                                    diag = (j == i // 4)
                                    nc.tensor.matmul(
                                        ps_s[:], qh[:, 128 * i:128 * (i + 1)],
                                        kh[:, 512 * j:512 * (j + 1)],
                                        start=True, stop=not diag)
                                    if diag:
                                        nc.tensor.matmul(
                                            ps_s[:], ident[:], cmask[i % 4][:],
                                            start=False, stop=True)
                                    se_j = small.tile([128, 1], F32, tag="se")
                                    nc.scalar.activation(
                                        pb[:, 512 * j:512 * (j + 1)], ps_s[:],
                                        Exp, scale=SCALE, accum_out=se_j[:])
                                    ses.append(se_j)
                                if len(ses) == 2:
                                    se = small.tile([128, 1], F32, tag="se2")
                                    nc.vector.tensor_add(
                                        se[:], ses[0][:], ses[1][:])
                                else:
                                    se = ses[0]
                                rse = small.tile([128, 1], F32, tag="rse")
                                nc.vector.reciprocal(rse[:], se[:])
                                # fold 1/sum into the transpose-matmul:
                                # pt = pb_chunk.T @ diag(rse) scales each
                                # q-token column by its 1/sum
                                drse = ap2.tile([128, 128], BF16, tag="drse",
                                                bufs=2)
                                nc.vector.tensor_scalar_mul(
                                    drse[:], ident[:], rse[:])
                                for tq in range(nv // 512):
                                    pt = psq.tile([128, 4, 128], F32,
                                                  tag="trf")
                                    for q in range(4):
                                        nc.tensor.matmul(
                                            pt[:, q, :],
                                            pb[:, 512 * tq + 128 * q:
                                               512 * tq + 128 * (q + 1)],
                                            drse[:], start=True, stop=True)
                                    nc.vector.tensor_copy(
                                        pT_sb[:, 4 * tq:4 * (tq + 1),
                                              128 * i:128 * (i + 1)], pt[:])
                            for ns in range(2):
                                po = psmm.tile([128, 512], F32, tag="mm")
                                kmax = 4 if ns == 0 else 8
                                for tcb in range(kmax):
                                    nc.tensor.matmul(
                                        po[:],
                                        v_b[:, tcb, 128 * hh:128 * (hh + 1)],
                                        pT_sb[:, tcb, 512 * ns:512 * (ns + 1)],
                                        start=(tcb == 0), stop=(tcb == kmax - 1))
                                nc.vector.tensor_copy(
                                    oT[:, hh, 512 * ns:512 * (ns + 1)], po[:])

                        # ---- o_proj for batch b -> rs_in_b -> RS_b -> AG_b
                        rs_in_b = dram.tile([NCORES * 128, D], BF16,
                                            tag=f"rsi_a{l}_{b}",
                                            name=f"rsi_a{l}_{b}")
                        for c8 in range(NCORES):
                            for n in range(4):
                                pp = psmm.tile([128, 512], F32, tag="mm")
                                for kc in range(2):
                                    nc.tensor.matmul(
                                        pp[:],
                                        oT[:, kc, 128 * c8:128 * (c8 + 1)],
                                        wo_sb[:, kc, 512 * n:512 * (n + 1)],
                                        start=(kc == 0), stop=(kc == 1))
                                ob = drain.tile([128, 512], BF16, tag="ob")
                                if n % 2 == 0:
                                    nc.vector.tensor_copy(ob[:], pp[:])
                                else:
                                    nc.scalar.copy(ob[:], pp[:])
                                nc.sync.dma_start(
                                    rs_in_b[128 * c8:128 * (c8 + 1),
                                            512 * n:512 * (n + 1)], ob[:])
                        rs_out_b = reduce_scatter_q(rs_in_b, f"a{l}_{b}")
                        pending[0] = make_post(b, rs_out_b, f"hct_m{l}_{b}",
                                               f"m{l}_{b}", hall_m)

                # ========================= MLP ==========================
                with tc.tile_pool(name=f"mlp{l}", bufs=1) as mp, \
                     tc.tile_pool(name=f"mlp2{l}", bufs=2) as mp2, \
                     tc.tile_pool(name=f"mlpa{l}", bufs=1) as maT:
                    wu_sb = mp.tile([128, KC, FFC], BF16, tag="wu")
                    nc.scalar.dma_start(wu_sb[:], wu_ext[l])
                    wd_sb = mp.tile([128, 8, D], BF16, tag="wd")
                    nc.scalar.dma_start(wd_sb[:], wd_ext[l])

                    hall_n = {}
                    for q in range(B):
                        h01 = h_next[0]
                        nxt = []
                        if q + 1 < B:
                            nxt_src = hall_m[q + 1]
                        elif l < L - 1:
                            nxt_src = hall_n[0]
                        else:
                            nxt_src = None
                        aT = maT.tile([128, 8, S], BF16, tag="aT")
                        for rr in range(2):
                            hh_sb = h01[rr]
                            for m in range(8):
                                pg = psmm.tile([128, 512], F32, tag="mm")
                                for kc in range(KC):
                                    nc.tensor.matmul(
                                        pg[:],
                                        wg_sb[:, kc, 128 * m:128 * (m + 1)],
                                        hh_sb[:, kc, :],
                                        start=(kc == 0), stop=(kc == KC - 1))
                                pu = psmm.tile([128, 512], F32, tag="mm")
                                for kc in range(KC):
                                    nc.tensor.matmul(
                                        pu[:],
                                        wu_sb[:, kc, 128 * m:128 * (m + 1)],
                                        hh_sb[:, kc, :],
                                        start=(kc == 0), stop=(kc == KC - 1))
                                sg_t = mp2.tile([128, 512], F32, tag="silu")
                                nc.scalar.activation(sg_t[:], pg[:], Sigmoid)
                                nc.vector.tensor_mul(sg_t[:], sg_t[:], pg[:])
                                nc.vector.tensor_mul(
                                    aT[:, m, 512 * rr:512 * (rr + 1)],
                                    sg_t[:], pu[:])
                            if nxt_src is not None:
                                nxt.append(load_h_half(nxt_src, rr))
                        if nxt_src is not None:
                            h_next[0] = nxt
                        flush()
                        last = (l == L - 1 and q == B - 1)
                        if last:
                            rs_in_h = [dram.tile([NCORES * 128, D // 2], BF16,
                                                 tag=f"rsi_m{l}_{q}_h{hh2}",
                                                 name=f"rsi_m{l}_{q}_h{hh2}")
                                       for hh2 in range(2)]
                        else:
                            rs_in_q = dram.tile([NCORES * 128, D], BF16,
                                                tag=f"rsi_m{l}_{q}",
                                                name=f"rsi_m{l}_{q}")
                        rs_out_h = []
                        for n in range(4):
                            for t in range(8):
                                pd = psmm.tile([128, 512], F32, tag="mm")
                                for kc in range(8):
                                    nc.tensor.matmul(
                                        pd[:],
                                        aT[:, kc, 128 * t:128 * (t + 1)],
                                        wd_sb[:, kc,
                                              512 * n:512 * (n + 1)],
                                        start=(kc == 0), stop=(kc == 7))
                                ob = drain.tile([128, 512], BF16, tag="ob")
                                if t % 2 == 0:
                                    nc.vector.tensor_copy(ob[:], pd[:])
                                else:
                                    nc.scalar.copy(ob[:], pd[:])
                                if last:
                                    nc.sync.dma_start(
                                        rs_in_h[n // 2][
                                            128 * t:128 * (t + 1),
                                            512 * (n % 2):512 * (n % 2 + 1)],
                                        ob[:])
                                else:
                                    nc.sync.dma_start(
                                        rs_in_q[128 * t:128 * (t + 1),
                                                512 * n:512 * (n + 1)], ob[:])
                            if last and n % 2 == 1:
                                rs_out_h.append(reduce_scatter_q(
                                    rs_in_h[n // 2], f"m{l}_{q}_h{n // 2}",
                                    cols=D // 2))
                        if last:
                            def final_post(rs_out_h=rs_out_h):
                                for hh2 in range(2):
                                    for n2 in range(2):
                                        db = drain.tile([128, 512], BF16,
                                                        tag="ob")
                                        nc.sync.dma_start(
                                            db[:],
                                            rs_out_h[hh2][:, 512 * n2:
                                                          512 * (n2 + 1)])
                                        lo = 1024 * hh2 + 512 * n2
                                        nc.vector.tensor_add(
                                            x_sb[:, q, lo:lo + 512],
                                            x_sb[:, q, lo:lo + 512], db[:])
                                final_tile(q)
                            pending[0] = final_post
                        else:
                            rs_out_q = reduce_scatter_q(rs_in_q, f"m{l}_{q}")
                            pending[0] = make_post(
                                q, rs_out_q, f"hct_a{l + 1}_{q}",
                                f"a{l + 1}_{q}",
                                hall_n if l < L - 1 else None)
                    hall = hall_n

            # ===================== final head (pool partials deferred) ====
            with tc.tile_pool(name="fin", bufs=1) as finp:
                flush()
                py = psq.tile([NS, 4], F32, tag="trf")
                for kc in range(KC):
                    nc.tensor.matmul(
                        py[:], hw_sb[:, kc, :], pooledT[:, kc, :],
                        start=(kc == 0), stop=(kc == KC - 1))
                y_sb = finp.tile([NS, 4], F32, tag="y")
                nc.vector.tensor_copy(y_sb[:], py[:])
                nc.sync.dma_start(out_ext[:], y_sb[:])

    nc.compile()
    return nc


# ---------------------------------------------------------------- host prep
def _dequant(codes, scales):
    """codes uint8 [..., O, I], scales f32 [..., O, I//BLK] -> f32 [..., O, I]."""
    w = NF4[codes]
    shp = w.shape
    w = w.reshape(shp[:-1] + (shp[-1] // BLK, BLK)) * scales[..., None]
    return w.reshape(shp).astype(np.float32)


def prep_inputs(inputs):
    """Full inputs -> per-core in_maps."""
    ii = np.asarray(inputs["input_ids"])
    embed = np.asarray(inputs["embed"], dtype=np.float32)
    x0 = embed[ii].reshape(TOK, D)

    attn_w = _dequant(np.asarray(inputs["attn_codes"]),
                      np.asarray(inputs["attn_scales"], dtype=np.float32))
    gu_w = _dequant(np.asarray(inputs["gu_codes"]),
                    np.asarray(inputs["gu_scales"], dtype=np.float32))
    down_w = _dequant(np.asarray(inputs["down_codes"]),
                      np.asarray(inputs["down_scales"], dtype=np.float32))
    anw = np.asarray(inputs["attn_norm_w"], dtype=np.float32)   # [L, D]
    mnw = np.asarray(inputs["mlp_norm_w"], dtype=np.float32)    # [L, D]
    fnw = np.asarray(inputs["final_norm_w"], dtype=np.float32)  # [D]
    head_w = np.asarray(inputs["head_w"], dtype=np.float32)     # [NS, D]

    # rope basis permutation within each head: [even dims, odd dims]
    perm = np.concatenate([np.arange(0, DH, 2), np.arange(1, DH, 2)])

    # rope tables over in-batch positions 0..S-1
    inv = 1.0 / (ROPE_THETA ** (np.arange(0, DH, 2, dtype=np.float32) / DH))
    ang = inv[:, None] * np.arange(S, dtype=np.float32)[None, :]    # [64, S]
    cosf = np.concatenate([np.cos(ang), np.cos(ang)], axis=0).astype(BF)
    sinf = np.concatenate([-np.sin(ang), np.sin(ang)], axis=0).astype(BF)

    # causal mask tiles: cmask[d, sl, tl] = 0 if 128*d + sl >= tl else -1e9
    sl = np.arange(128)[:, None]
    tl = np.arange(512)[None, :]
    cmask = np.stack([np.where(128 * d + sl >= tl, 0.0, -1e9)
                      for d in range(4)]).astype(BF)

    hwp = (head_w * fnw[None, :]).T.copy().astype(np.float32)       # [D, NS]

    # replicated, pre-normed + transposed layer-0 hidden (hall layout):
    # hall0[b, 128*rg + p, 128*c + n] = hnorm[b, 128*rg + n, 128*c + p]
    xb = x0.reshape(B, S, D)
    hn = xb * (1.0 / np.sqrt(np.mean(xb * xb, axis=-1, keepdims=True) + EPS))
    hall0 = np.ascontiguousarray(
        hn.reshape(B, 8, 128, KC, 128).transpose(0, 1, 4, 3, 2)
          .reshape(B, NCORES * 128, D)).astype(BF)

    in_maps = []
    for c in range(NCORES):
        m = {}
        m["hall0"] = hall0
        # strided token shard: batch-tile t = tokens [1024*t + 128*c, +128)
        xrows = np.concatenate([x0[1024 * t + 128 * c:1024 * t + 128 * (c + 1)]
                                for t in range(4)])
        m["x"] = np.ascontiguousarray(xrows)
        rows = slice(DQ * c, DQ * (c + 1))
        prows = np.concatenate([perm + DH * h for h in (0, 1)]) + DQ * c
        # fold input-side rmsnorm weight into the projection weights
        wq = np.stack([(attn_w[l, 0][prows] * anw[l][None, :]).T for l in range(L)])
        wk = np.stack([(attn_w[l, 1][prows] * anw[l][None, :]).T for l in range(L)])
        wv = np.stack([(attn_w[l, 2][rows] * anw[l][None, :]).T for l in range(L)])
        wo = np.stack([attn_w[l, 3][:, rows].T for l in range(L)])
        frows = slice(FFC * c, FFC * (c + 1))
        wg = np.stack([(gu_w[l, 0][frows] * mnw[l][None, :]).T for l in range(L)])
        wu = np.stack([(gu_w[l, 1][frows] * mnw[l][None, :]).T for l in range(L)])
        wd = np.stack([down_w[l][:, frows].T for l in range(L)])
        def pmajor(w):
            # [L, nchunk*128, cols] -> [L, 128, nchunk*cols]
            Lw, rows, cols = w.shape
            nch = rows // 128
            return np.ascontiguousarray(
                w.reshape(Lw, nch, 128, cols).transpose(0, 2, 1, 3)
                 .reshape(Lw, 128, nch * cols))
        for k, wmat in (("wq", wq), ("wk", wk), ("wv", wv), ("wo", wo),
                        ("wg", wg), ("wu", wu), ("wd", wd)):
            m[k] = pmajor(np.ascontiguousarray(wmat)).astype(BF)
        m["cosf"] = cosf
        m["sinf"] = sinf
        m["cmask"] = cmask
        m["hw"] = hwp
        in_maps.append(m)
    return in_maps


def kernel(**inputs):
    from concourse.bass_utils import run_bass_kernel_spmd

    if "nc" not in _CACHE:
        _CACHE["nc"] = build_graph()
    nc = _CACHE["nc"]
    in_maps = prep_inputs(inputs)
    res = run_bass_kernel_spmd(nc, in_maps, core_ids=list(range(NCORES)))
    head_b = np.asarray(inputs["head_b"], dtype=np.float32)
    out = sum(res.results[c]["out"] for c in range(NCORES)).T  # [B, NS]
    return (out + head_b[None, :]).astype(np.float32)


# revision 38
# speedup vs baseline: 1.0111x; 1.0111x over previous
"""Self-contained TRN2 Bass kernel for nn_AESModel_42760694399363.

2-layer NF4-quantized transformer (B=4,S=1024,D=2048,FF=8192,H=16) + mean-pool
+ linear head.  Tensor-parallel across 8 NeuronCores: q/k/v/gate/up
column-sharded, o/down row-sharded.  The residual stream is sequence-sharded
STRIDED BY BATCH: core c owns tokens {1024*q + 128*c .. +128} for each batch
q.  This makes every boundary collective a per-batch 4MB ReduceScatter /
AllGather that pipelines behind the next batch's attention (or next batch's
MLP chunk), hiding nearly all collective latency.  Host does embedding gather
and NF4 dequant; all matmul/attention/norm FLOPs run on device in bf16 with
f32 accumulation.
"""
import sys

sys.path.insert(0, "/opt/trn_rl_repo")

import numpy as np
import ml_dtypes

# ---------------------------------------------------------------- constants
B, S, D, L, FF, V, NS = 4, 1024, 2048, 2, 8192, 32000, 11
H, DH = 16, 128
NCORES = 8
TOK = B * S              # 4096
SHARD = TOK // NCORES    # 512 tokens per core (4 tiles of 128, one per batch)
DQ = D // NCORES         # 256  q/k/v out-cols per core
FFC = FF // NCORES       # 1024 gate/up cols per core
KC = D // 128            # 16 contraction chunks over D
SCALE = 1.0 / np.sqrt(DH)
EPS = 1e-5
ROPE_THETA = 10000.0
BLK = 64

NF4 = np.array([-1.0, -0.6961928009986877, -0.5250730514526367, -0.39491748809814453,
                -0.28444138169288635, -0.18477343022823334, -0.09105003625154495, 0.0,
                0.07958029955625534, 0.16093020141124725, 0.24611230194568634,
                0.33791524171829224, 0.44070982933044434, 0.5626170039176941,
                0.7229568362236023, 1.0], dtype=np.float32)

BF = ml_dtypes.bfloat16

_CACHE = {}


# ---------------------------------------------------------------- device graph
def build_graph():
    import concourse.mybir as mybir
    import concourse.tile as tile
    from concourse import bacc
    from concourse.masks import make_identity
    from concourse.tile_rust import add_dep_helper

    F32 = mybir.dt.float32
    BF16 = mybir.dt.bfloat16
    RG = [list(range(NCORES))]
    Exp = mybir.ActivationFunctionType.Exp
    Sigmoid = mybir.ActivationFunctionType.Sigmoid

    nc = bacc.Bacc("TRN2", target_bir_lowering=False, debug=False,
                   num_devices=NCORES)
    _prev_cc = [None]

    def _chain_cc(cc):
        # never allow two collectives in flight: serialize in issue order
        if _prev_cc[0] is not None:
            add_dep_helper(cc.ins, _prev_cc[0], reason="serialize collectives")
        _prev_cc[0] = cc.ins

    x_ext = nc.declare_dram_parameter("x", [SHARD, D], F32, isOutput=False)
    hall0_ext = nc.declare_dram_parameter("hall0", [B, NCORES * 128, D], BF16,
                                          isOutput=False)
    wq_ext = nc.declare_dram_parameter("wq", [L, 128, KC * DQ], BF16, isOutput=False)
    wk_ext = nc.declare_dram_parameter("wk", [L, 128, KC * DQ], BF16, isOutput=False)
    wv_ext = nc.declare_dram_parameter("wv", [L, 128, KC * DQ], BF16, isOutput=False)
    wo_ext = nc.declare_dram_parameter("wo", [L, 128, 2 * D], BF16, isOutput=False)
    wg_ext = nc.declare_dram_parameter("wg", [L, 128, KC * FFC], BF16, isOutput=False)
    wu_ext = nc.declare_dram_parameter("wu", [L, 128, KC * FFC], BF16, isOutput=False)
    wd_ext = nc.declare_dram_parameter("wd", [L, 128, 8 * D], BF16, isOutput=False)
    cosf_ext = nc.declare_dram_parameter("cosf", [128, S], BF16, isOutput=False)
    sinf_ext = nc.declare_dram_parameter("sinf", [128, S], BF16, isOutput=False)
    cm_ext = nc.declare_dram_parameter("cmask", [4, 128, 512], BF16, isOutput=False)
    hw_ext = nc.declare_dram_parameter("hw", [D, NS], F32, isOutput=False)
    out_ext = nc.declare_dram_parameter("out", [NS, B], F32, isOutput=True)

    with tile.TileContext(nc) as tc:
        with tc.tile_pool(name="const", bufs=1) as constp, \
             tc.tile_pool(name="xres", bufs=1) as xres, \
             tc.tile_pool(name="norm", bufs=1) as normp, \
             tc.tile_pool(name="wpre", bufs=1) as wpre, \
             tc.tile_pool(name="hpool", bufs=2) as hpool, \
             tc.tile_pool(name="small", bufs=4) as small, \
             tc.tile_pool(name="drain", bufs=8) as drain, \
             tc.tile_pool(name="psmm", bufs=5, space="PSUM") as psmm, \
             tc.tile_pool(name="pstr", bufs=2, space="PSUM") as pstr, \
             tc.tile_pool(name="psq", bufs=1, space="PSUM") as psq, \
             tc.tile_pool(name="dram", bufs=1, space="DRAM") as dram:

            warm_in = dram.tile([8, 8], BF16, tag="ccwarm_in", name="ccwarm_in")
            warm_out = dram.tile([64, 8], BF16, tag="ccwarm_out",
                                 name="ccwarm_out", addr_space="Shared")
            cc0 = nc.gpsimd.collective_compute(
                "AllGather", mybir.AluOpType.bypass,
                replica_groups=[list(range(NCORES))],
                ins=[warm_in.opt()], outs=[warm_out.opt()])
            _chain_cc(cc0)
            ident = constp.tile([128, 128], BF16, tag="ident")
            make_identity(nc, ident[:])
            hw_sb = constp.tile([128, KC, NS], F32, tag="hw")
            nc.scalar.dma_start(
                hw_sb[:], hw_ext.ap().rearrange("(c p) m -> p c m", p=128))
            pvec = constp.tile([128, 128], BF16, tag="pvec")
            nc.vector.memset(pvec[:], 1.0 / S)
            pooled1 = constp.tile([1, D], F32, tag="pooled1")
            pooledT = constp.tile([128, KC, 4], F32, tag="pooledT")

            # residual stream, f32, [128 part, q=batch tile, D]
            x_sb = xres.tile([128, 4, D], F32, tag="x")

            def rmsnorm_bf16(t, h_out):
                """h_out[128, D] bf16 = x_sb[:, t, :] * rsqrt(mean(x^2)+eps)."""
                ssq = small.tile([128, 1], F32, tag="ssq")
                nc.scalar.activation(h_out[:], x_sb[:, t, :],
                                     mybir.ActivationFunctionType.Square,
                                     accum_out=ssq[:])
                ms = small.tile([128, 1], F32, tag="ms")
                nc.vector.tensor_scalar(ms[:], ssq[:], 1.0 / D, EPS,
                                        mybir.AluOpType.mult,
                                        mybir.AluOpType.add)
                st = small.tile([128, 1], F32, tag="st")
                nc.scalar.sqrt(st[:], ms[:])
                rstd = small.tile([128, 1], F32, tag="rstd")
                nc.vector.reciprocal(rstd[:], st[:])
                nc.vector.tensor_scalar_mul(h_out[:], x_sb[:, t, :], rstd[:])

            def norm_transpose(t, hct_q):
                """rmsnorm batch-tile t -> PE transposes -> hct_q [128, (c n)]."""
                hbf = normp.tile([128, D], BF16, tag="hbf")
                rmsnorm_bf16(t, hbf)
                for kq in range(KC // 4):
                    pt = pstr.tile([128, 4, 128], BF16, tag="tr")
                    for q in range(4):
                        nc.tensor.transpose(
                            pt[:, q, :],
                            hbf[:, 512 * kq + 128 * q:512 * kq + 128 * (q + 1)],
                            ident[:])
                    hts = drain.tile([128, 4, 128], BF16, tag="ob")
                    nc.vector.tensor_copy(hts[:], pt[:])
                    nc.sync.dma_start(
                        hct_q[:, 512 * kq:512 * (kq + 1)], hts[:])

            def allgather_q(hct_q, tagsuf):
                """AG [128, D] -> [NCORES*128, D] (4MB)."""
                hall_q = dram.tile([NCORES * 128, D], BF16, tag="hall" + tagsuf,
                                   name="hall" + tagsuf, addr_space="Shared")
                cc = nc.gpsimd.collective_compute(
                    "AllGather", mybir.AluOpType.bypass, replica_groups=RG,
                    ins=[hct_q.opt()], outs=[hall_q.opt()])
                _chain_cc(cc)
                return hall_q

            def reduce_scatter_q(rs_in_q, tagsuf, cols=D):
                """RS [8*128, cols] -> [128, cols]."""
                rs_out_q = dram.tile([128, cols], BF16, tag="rso" + tagsuf,
                                     name="rso" + tagsuf)
                cc = nc.gpsimd.collective_compute(
                    "ReduceScatter", mybir.AluOpType.add, replica_groups=RG,
                    ins=[rs_in_q.opt()], outs=[rs_out_q.opt()])
                _chain_cc(cc)
                return rs_out_q

            def residual_add(t, rs_out_q):
                """x_sb[:, t, :] += rs_out_q [128, D] (bf16 in DRAM)."""
                for n in range(4):
                    db = drain.tile([128, 512], BF16, tag="ob")
                    nc.sync.dma_start(
                        db[:], rs_out_q[:, 512 * n:512 * (n + 1)])
                    nc.vector.tensor_add(
                        x_sb[:, t, 512 * n:512 * (n + 1)],
                        x_sb[:, t, 512 * n:512 * (n + 1)], db[:])

            def load_h_half(hall_q, rr):
                """Half of a batch's gathered h: [128, KC, 512] from hall_q."""
                h_sb = hpool.tile([128, KC, 512], BF16, tag="hr")
                for r in range(4):
                    rg = 4 * rr + r
                    nc.sync.dma_start(
                        h_sb[:, :, 128 * r:128 * (r + 1)],
                        hall_q[128 * rg:128 * (rg + 1), :].rearrange(
                            "p (c n) -> p c n", n=128))
                return h_sb

            def load_h_pair(hall_q):
                return [load_h_half(hall_q, 0), load_h_half(hall_q, 1)]

            pending = [None]

            def flush():
                if pending[0] is not None:
                    f = pending[0]
                    pending[0] = None
                    f()

            def make_post(b, rs_out_q, hct_name, agsuf, store):
                """Deferred post-RS work: residual add (+ norm/transpose/AG)."""
                def run():
                    residual_add(b, rs_out_q)
                    if store is not None:
                        hct_q = dram.tile([128, D], BF16, tag=hct_name,
                                          name=hct_name)
                        norm_transpose(b, hct_q)
                        store[b] = allgather_q(hct_q, agsuf)
                    else:
                        final_tile(b)
                return run

            def final_tile(t):
                """Final rmsnorm + mean-pool partial for batch tile t."""
                hfin = normp.tile([128, D], BF16, tag="hbf")
                rmsnorm_bf16(t, hfin)
                for n in range(4):
                    pq = psq.tile([128, 512], F32, tag="pool",
                                  name=f"pool{t}_{n}")
                    nc.tensor.matmul(pq[:], pvec[:],
                                     hfin[:, 512 * n:512 * (n + 1)],
                                     start=True, stop=True)
                    nc.vector.tensor_copy(
                        pooled1[0:1, 512 * n:512 * (n + 1)], pq[0:1, :])
                pdram_t = dram.tile([1, D], F32, tag=f"pd{t}", name=f"pd{t}")
                nc.sync.dma_start(pdram_t[:], pooled1[:])
                nc.sync.dma_start(
                    pooledT[:, :, t],
                    pdram_t[0:1, :].rearrange("a (c p) -> p (a c)", p=128))

            # ---------------- initial hidden state comes pre-normed,
            # pre-transposed and replicated from the host (hall0): no
            # initial AllGather chain on the critical path.
            hall = {t: hall0_ext[t] for t in range(4)}
            h_next = [load_h_pair(hall[0])]

            for l in range(L):
                # ======================= attention =======================
                with tc.tile_pool(name=f"attn{l}", bufs=1) as ap, \
                     tc.tile_pool(name=f"attn2{l}", bufs=1) as ap2, \
                     tc.tile_pool(name=f"attnpT{l}", bufs=1) as appt:
                    wq_sb = ap.tile([128, KC, DQ], BF16, tag="wq")
                    nc.scalar.dma_start(wq_sb[:], wq_ext[l])
                    wk_sb = ap.tile([128, KC, DQ], BF16, tag="wk")
                    nc.scalar.dma_start(wk_sb[:], wk_ext[l])
                    wv_sb = ap.tile([128, KC, DQ], BF16, tag="wv")
                    nc.scalar.dma_start(wv_sb[:], wv_ext[l])
                    cosf = ap.tile([128, S], BF16, tag="cosf")
                    nc.scalar.dma_start(cosf[:], cosf_ext[:])
                    sinf = ap.tile([128, S], BF16, tag="sinf")
                    nc.scalar.dma_start(sinf[:], sinf_ext[:])
                    cmask = [ap.tile([128, 512], BF16, tag=f"cmask{d}",
                                     name=f"cmask{l}_{d}") for d in range(4)]
                    for d in range(4):
                        nc.scalar.dma_start(cmask[d][:], cm_ext[d])
                    # prefetch next MLP's gate weights during attention
                    wg_sb = wpre.tile([128, KC, FFC], BF16, tag="wg")
                    nc.scalar.dma_start(wg_sb[:], wg_ext[l])
                    wo_sb = ap.tile([128, 2, D], BF16, tag="wo")
                    nc.scalar.dma_start(wo_sb[:], wo_ext[l])
                    if l == 0:
                        nc.sync.dma_start(
                            x_sb[:],
                            x_ext.ap().rearrange("(t p) d -> p t d", p=128))

                    hall_m = {}
                    for b in range(B):
                        h01 = h_next[0]
                        # ---- qkv for this batch (1024 tokens, 2 halves)
                        qTb = ap.tile([128, 2, S], BF16, tag="qTb")
                        kTb = ap.tile([128, 2, S], BF16, tag="kTb")
                        v_b = ap.tile([128, 8, DQ], BF16, tag="vb")
                        for rr in range(2):
                            hh_sb = h01[rr]
                            cs = cosf[:, 512 * rr:512 * (rr + 1)]
                            sn = sinf[:, 512 * rr:512 * (rr + 1)]
                            for wsb, dstT in ((wq_sb, qTb), (wk_sb, kTb)):
                                for m in range(2):
                                    pq = psmm.tile([128, 512], F32, tag="mm")
                                    for kc in range(KC):
                                        nc.tensor.matmul(
                                            pq[:],
                                            wsb[:, kc, 128 * m:128 * (m + 1)],
                                            hh_sb[:, kc, :],
                                            start=(kc == 0), stop=(kc == KC - 1))
                                    ta = ap2.tile([128, 512], F32, tag="ropea")
                                    nc.vector.tensor_mul(ta[:], pq[:], cs)
                                    tb = ap2.tile([128, 512], F32, tag="ropeb")
                                    nc.vector.tensor_mul(
                                        tb[0:64, :], pq[64:128, :], sn[0:64, :])
                                    nc.vector.tensor_mul(
                                        tb[64:128, :], pq[0:64, :],
                                        sn[64:128, :])
                                    nc.vector.tensor_add(
                                        dstT[:, m, 512 * rr:512 * (rr + 1)],
                                        ta[:], tb[:])
                            for t in range(4):
                                pv = psmm.tile([128, DQ], F32, tag="mm")
                                for kc in range(KC):
                                    nc.tensor.matmul(
                                        pv[:],
                                        hh_sb[:, kc, 128 * t:128 * (t + 1)],
                                        wv_sb[:, kc, :],
                                        start=(kc == 0), stop=(kc == KC - 1))
                                nc.vector.tensor_copy(
                                    v_b[:, 4 * rr + t, :], pv[:])
                        if b + 1 < B:
                            h_next[0] = load_h_pair(hall[b + 1])
                        else:
                            h_next[0] = load_h_pair(hall_m[0])
                        # ---- attention for b, per local head
                        oT = ap.tile([128, 2, S], BF16, tag="oT")
                        for hh in range(2):
                            if hh == 1:
                                flush()
                            qh = qTb[:, hh, :]
                            kh = kTb[:, hh, :]
                            pT_sb = appt.tile([128, 8, S], BF16, tag="pT")
                            for i in range(8):
                                nv = 512 if i < 4 else 1024
                                # scores stay in PSUM; causal mask accumulated
                                # by the PE (ident.T @ cmask); exp reads PSUM.
                                # No max-subtraction: |SCALE*s| << 80 for this
                                # model, so plain exp is exact & overflow-free.
                                pb = ap2.tile([128, S], BF16, tag="pb", bufs=3)
                                ses = []
                                for j in range(nv // 512):
                                    ps_s = psmm.tile([128, 512], F32, tag="mm")
                                    diag = (j == i // 4)
                                    nc.tensor.matmul(
                                        ps_s[:], qh[:, 128 * i:128 * (i + 1)],
                                        kh[:, 512 * j:512 * (j + 1)],
                                        start=True, stop=not diag)
                                    if diag:
                                        nc.tensor.matmul(
                                            ps_s[:], ident[:], cmask[i % 4][:],
                                            start=False, stop=True)
                                    se_j = small.tile([128, 1], F32, tag="se")
                                    nc.scalar.activation(
                                        pb[:, 512 * j:512 * (j + 1)], ps_s[:],
                                        Exp, scale=SCALE, accum_out=se_j[:])
                                    ses.append(se_j)
                                if len(ses) == 2:
                                    se = small.tile([128, 1], F32, tag="se2")
                                    nc.vector.tensor_add(
                                        se[:], ses[0][:], ses[1][:])
                                else:
                                    se = ses[0]
                                rse = small.tile([128, 1], F32, tag="rse")
                                nc.vector.reciprocal(rse[:], se[:])
                                nc.vector.tensor_scalar_mul(
                                    pb[:, :nv], pb[:, :nv], rse[:])
                                for tq in range(nv // 512):
                                    pt = pstr.tile([128, 4, 128], BF16,
                                                   tag="tr")
                                    for q in range(4):
                                        nc.tensor.transpose(
                                            pt[:, q, :],
                                            pb[:, 512 * tq + 128 * q:
                                               512 * tq + 128 * (q + 1)],
                                            ident[:])
                                    nc.vector.tensor_copy(
                                        pT_sb[:, 4 * tq:4 * (tq + 1),
                                              128 * i:128 * (i + 1)], pt[:])
                            for ns in range(2):
                                po = psmm.tile([128, 512], F32, tag="mm")
                                kmax = 4 if ns == 0 else 8
                                for tcb in range(kmax):
                                    nc.tensor.matmul(
                                        po[:],
                                        v_b[:, tcb, 128 * hh:128 * (hh + 1)],
                                        pT_sb[:, tcb, 512 * ns:512 * (ns + 1)],
                                        start=(tcb == 0), stop=(tcb == kmax - 1))
                                nc.vector.tensor_copy(
                                    oT[:, hh, 512 * ns:512 * (ns + 1)], po[:])

                        # ---- o_proj for batch b -> rs_in_b -> RS_b -> AG_b
                        rs_in_b = dram.tile([NCORES * 128, D], BF16,
                                            tag=f"rsi_a{l}_{b}",
                                            name=f"rsi_a{l}_{b}")
                        for c8 in range(NCORES):
                            for n in range(4):
                                pp = psmm.tile([128, 512], F32, tag="mm")
                                for kc in range(2):
                                    nc.tensor.matmul(
                                        pp[:],
                                        oT[:, kc, 128 * c8:128 * (c8 + 1)],
                                        wo_sb[:, kc, 512 * n:512 * (n + 1)],
                                        start=(kc == 0), stop=(kc == 1))
                                ob = drain.tile([128, 512], BF16, tag="ob")
                                if n % 2 == 0:
                                    nc.vector.tensor_copy(ob[:], pp[:])
                                else:
                                    nc.scalar.copy(ob[:], pp[:])
                                nc.sync.dma_start(
                                    rs_in_b[128 * c8:128 * (c8 + 1),
                                            512 * n:512 * (n + 1)], ob[:])
                        rs_out_b = reduce_scatter_q(rs_in_b, f"a{l}_{b}")
                        pending[0] = make_post(b, rs_out_b, f"hct_m{l}_{b}",
                                               f"m{l}_{b}", hall_m)

                # ========================= MLP ==========================
                with tc.tile_pool(name=f"mlp{l}", bufs=1) as mp, \
                     tc.tile_pool(name=f"mlp2{l}", bufs=2) as mp2, \
                     tc.tile_pool(name=f"mlpa{l}", bufs=1) as maT:
                    wu_sb = mp.tile([128, KC, FFC], BF16, tag="wu")
                    nc.scalar.dma_start(wu_sb[:], wu_ext[l])
                    wd_sb = mp.tile([128, 8, D], BF16, tag="wd")
                    nc.scalar.dma_start(wd_sb[:], wd_ext[l])

                    hall_n = {}
                    for q in range(B):
                        h01 = h_next[0]
                        nxt = []
                        if q + 1 < B:
                            nxt_src = hall_m[q + 1]
                        elif l < L - 1:
                            nxt_src = hall_n[0]
                        else:
                            nxt_src = None
                        aT = maT.tile([128, 8, S], BF16, tag="aT")
                        for rr in range(2):
                            hh_sb = h01[rr]
                            for m in range(8):
                                pg = psmm.tile([128, 512], F32, tag="mm")
                                for kc in range(KC):
                                    nc.tensor.matmul(
                                        pg[:],
                                        wg_sb[:, kc, 128 * m:128 * (m + 1)],
                                        hh_sb[:, kc, :],
                                        start=(kc == 0), stop=(kc == KC - 1))
                                pu = psmm.tile([128, 512], F32, tag="mm")
                                for kc in range(KC):
                                    nc.tensor.matmul(
                                        pu[:],
                                        wu_sb[:, kc, 128 * m:128 * (m + 1)],
                                        hh_sb[:, kc, :],
                                        start=(kc == 0), stop=(kc == KC - 1))
                                sg_t = mp2.tile([128, 512], F32, tag="silu")
                                nc.scalar.activation(sg_t[:], pg[:], Sigmoid)
                                nc.vector.tensor_mul(sg_t[:], sg_t[:], pg[:])
                                nc.vector.tensor_mul(
                                    aT[:, m, 512 * rr:512 * (rr + 1)],
                                    sg_t[:], pu[:])
                            if nxt_src is not None:
                                nxt.append(load_h_half(nxt_src, rr))
                        if nxt_src is not None:
                            h_next[0] = nxt
                        flush()
                        last = (l == L - 1 and q == B - 1)
                        if last:
                            rs_in_h = [dram.tile([NCORES * 128, D // 2], BF16,
                                                 tag=f"rsi_m{l}_{q}_h{hh2}",
                                                 name=f"rsi_m{l}_{q}_h{hh2}")
                                       for hh2 in range(2)]
                        else:
                            rs_in_q = dram.tile([NCORES * 128, D], BF16,
                                                tag=f"rsi_m{l}_{q}",
                                                name=f"rsi_m{l}_{q}")
                        rs_out_h = []
                        for n in range(4):
                            for t in range(8):
                                pd = psmm.tile([128, 512], F32, tag="mm")
                                for kc in range(8):
                                    nc.tensor.matmul(
                                        pd[:],
                                        aT[:, kc, 128 * t:128 * (t + 1)],
                                        wd_sb[:, kc,
                                              512 * n:512 * (n + 1)],
                                        start=(kc == 0), stop=(kc == 7))
                                ob = drain.tile([128, 512], BF16, tag="ob")
                                if t % 2 == 0:
                                    nc.vector.tensor_copy(ob[:], pd[:])
                                else:
                                    nc.scalar.copy(ob[:], pd[:])
                                if last:
                                    nc.sync.dma_start(
                                        rs_in_h[n // 2][
                                            128 * t:128 * (t + 1),
                                            512 * (n % 2):512 * (n % 2 + 1)],
                                        ob[:])
                                else:
                                    nc.sync.dma_start(
                                        rs_in_q[128 * t:128 * (t + 1),
                                                512 * n:512 * (n + 1)], ob[:])
                            if last and n % 2 == 1:
                                rs_out_h.append(reduce_scatter_q(
                                    rs_in_h[n // 2], f"m{l}_{q}_h{n // 2}",
                                    cols=D // 2))
                        if last:
                            def final_post(rs_out_h=rs_out_h):
                                for hh2 in range(2):
                                    for n2 in range(2):
                                        db = drain.tile([128, 512], BF16,
                                                        tag="ob")
                                        nc.sync.dma_start(
                                            db[:],
                                            rs_out_h[hh2][:, 512 * n2:
                                                          512 * (n2 + 1)])
                                        lo = 1024 * hh2 + 512 * n2
                                        nc.vector.tensor_add(
                                            x_sb[:, q, lo:lo + 512],
                                            x_sb[:, q, lo:lo + 512], db[:])
                                final_tile(q)
                            pending[0] = final_post
                        else:
                            rs_out_q = reduce_scatter_q(rs_in_q, f"m{l}_{q}")
                            pending[0] = make_post(
                                q, rs_out_q, f"hct_a{l + 1}_{q}",
                                f"a{l + 1}_{q}",
                                hall_n if l < L - 1 else None)
                    hall = hall_n

            # ===================== final head (pool partials deferred) ====
            with tc.tile_pool(name="fin", bufs=1) as finp:
                flush()
                py = psq.tile([NS, 4], F32, tag="pool")
                for kc in range(KC):
                    nc.tensor.matmul(
                        py[:], hw_sb[:, kc, :], pooledT[:, kc, :],
                        start=(kc == 0), stop=(kc == KC - 1))
                y_sb = finp.tile([NS, 4], F32, tag="y")
                nc.vector.tensor_copy(y_sb[:], py[:])
                nc.sync.dma_start(out_ext[:], y_sb[:])

    nc.compile()
    return nc


# ---------------------------------------------------------------- host prep
def _dequant(codes, scales):
    """codes uint8 [..., O, I], scales f32 [..., O, I//BLK] -> f32 [..., O, I]."""
    w = NF4[codes]
    shp = w.shape
    w = w.reshape(shp[:-1] + (shp[-1] // BLK, BLK)) * scales[..., None]
    return w.reshape(shp).astype(np.float32)


def prep_inputs(inputs):
    """Full inputs -> per-core in_maps."""
    ii = np.asarray(inputs["input_ids"])
    embed = np.asarray(inputs["embed"], dtype=np.float32)
    x0 = embed[ii].reshape(TOK, D)

    attn_w = _dequant(np.asarray(inputs["attn_codes"]),
                      np.asarray(inputs["attn_scales"], dtype=np.float32))
    gu_w = _dequant(np.asarray(inputs["gu_codes"]),
                    np.asarray(inputs["gu_scales"], dtype=np.float32))
    down_w = _dequant(np.asarray(inputs["down_codes"]),
                      np.asarray(inputs["down_scales"], dtype=np.float32))
    anw = np.asarray(inputs["attn_norm_w"], dtype=np.float32)   # [L, D]
    mnw = np.asarray(inputs["mlp_norm_w"], dtype=np.float32)    # [L, D]
    fnw = np.asarray(inputs["final_norm_w"], dtype=np.float32)  # [D]
    head_w = np.asarray(inputs["head_w"], dtype=np.float32)     # [NS, D]

    # rope basis permutation within each head: [even dims, odd dims]
    perm = np.concatenate([np.arange(0, DH, 2), np.arange(1, DH, 2)])

    # rope tables over in-batch positions 0..S-1
    inv = 1.0 / (ROPE_THETA ** (np.arange(0, DH, 2, dtype=np.float32) / DH))
    ang = inv[:, None] * np.arange(S, dtype=np.float32)[None, :]    # [64, S]
    cosf = np.concatenate([np.cos(ang), np.cos(ang)], axis=0).astype(BF)
    sinf = np.concatenate([-np.sin(ang), np.sin(ang)], axis=0).astype(BF)

    # causal mask tiles: cmask[d, sl, tl] = 0 if 128*d + sl >= tl else -1e9
    sl = np.arange(128)[:, None]
    tl = np.arange(512)[None, :]
    cmask = np.stack([np.where(128 * d + sl >= tl, 0.0, -1e9)
                      for d in range(4)]).astype(BF)

    hwp = (head_w * fnw[None, :]).T.copy().astype(np.float32)       # [D, NS]

    # replicated, pre-normed + transposed layer-0 hidden (hall layout):
    # hall0[b, 128*rg + p, 128*c + n] = hnorm[b, 128*rg + n, 128*c + p]
    xb = x0.reshape(B, S, D)
    hn = xb * (1.0 / np.sqrt(np.mean(xb * xb, axis=-1, keepdims=True) + EPS))
    hall0 = np.ascontiguousarray(
        hn.reshape(B, 8, 128, KC, 128).transpose(0, 1, 4, 3, 2)
          .reshape(B, NCORES * 128, D)).astype(BF)

    in_maps = []
    for c in range(NCORES):
        m = {}
        m["hall0"] = hall0
        # strided token shard: batch-tile t = tokens [1024*t + 128*c, +128)
        xrows = np.concatenate([x0[1024 * t + 128 * c:1024 * t + 128 * (c + 1)]
                                for t in range(4)])
        m["x"] = np.ascontiguousarray(xrows)
        rows = slice(DQ * c, DQ * (c + 1))
        prows = np.concatenate([perm + DH * h for h in (0, 1)]) + DQ * c
        # fold input-side rmsnorm weight into the projection weights
        wq = np.stack([(attn_w[l, 0][prows] * anw[l][None, :]).T for l in range(L)])
        wk = np.stack([(attn_w[l, 1][prows] * anw[l][None, :]).T for l in range(L)])
        wv = np.stack([(attn_w[l, 2][rows] * anw[l][None, :]).T for l in range(L)])
        wo = np.stack([attn_w[l, 3][:, rows].T for l in range(L)])
        frows = slice(FFC * c, FFC * (c + 1))
        wg = np.stack([(gu_w[l, 0][frows] * mnw[l][None, :]).T for l in range(L)])
        wu = np.stack([(gu_w[l, 1][frows] * mnw[l][None, :]).T for l in range(L)])
        wd = np.stack([down_w[l][:, frows].T for l in range(L)])
        def pmajor(w):
            # [L, nchunk*128, cols] -> [L, 128, nchunk*cols]
            Lw, rows, cols = w.shape
            nch = rows // 128
            return np.ascontiguousarray(
                w.reshape(Lw, nch, 128, cols).transpose(0, 2, 1, 3)
                 .reshape(Lw, 128, nch * cols))
        for k, wmat in (("wq", wq), ("wk", wk), ("wv", wv), ("wo", wo),
                        ("wg", wg), ("wu", wu), ("wd", wd)):
            m[k] = pmajor(np.ascontiguousarray(wmat)).astype(BF)
        m["cosf"] = cosf
        m["sinf"] = sinf
        m["cmask"] = cmask
        m["hw"] = hwp
        in_maps.append(m)
    return in_maps


def kernel(**inputs):
    from concourse.bass_utils import run_bass_kernel_spmd

    if "nc" not in _CACHE:
        _CACHE["nc"] = build_graph()
    nc = _CACHE["nc"]
    in_maps = prep_inputs(inputs)
    # Warm-up execution: the very first run of a freshly loaded NEFF has
    # (rarely) produced corrupted collective results on this setup; repeat
    # executions are reliably correct.  Run once, discard, run again.
    if "warm" not in _CACHE:
        run_bass_kernel_spmd(nc, in_maps, core_ids=list(range(NCORES)))
        _CACHE["warm"] = True
    res = run_bass_kernel_spmd(nc, in_maps, core_ids=list(range(NCORES)))
    head_b = np.asarray(inputs["head_b"], dtype=np.float32)
    out = sum(res.results[c]["out"] for c in range(NCORES)).T  # [B, NS]
    return (out + head_b[None, :]).astype(np.float32)
